# revision 26
# baseline (speedup 1.0000x reference)
"""GATv2 layer on 8 Trainium2 NeuronCores (Bass/Tile).

Sharding (edge-parallel by destination range, per the problem's hint: each
device holds its edge shard plus gathered src/dst node features):
  - Device d owns destination nodes [d*6250, (d+1)*6250).
  - Host sorts edges by dst, routes each edge to the device owning its dst,
    and materializes the gathered node features feat[src], feat[dst] per edge
    slot (fp16) — these are the per-device inputs, so the device streams
    contiguous data instead of doing random-access gathers.
  - Per device, dst nodes are grouped into 49 windows of 128 nodes. Each
    window's incoming edges are padded to K tiles of 128 edges (K = global
    max, data-driven) so every core runs the identical SPMD program.
  - Per 512-edge group, transpose-DMA loads stream featT tiles; PE matmuls
    compute fs = feat_se @ W_src.T and x = fs + fd directly in PSUM;
    ACT applies LeakyReLU; DVE forms score = sum_f attn*z per head;
    ACT exponentiates (no max subtraction: scores are O(1) by construction
    and softmax is shift-invariant, matching the reference exactly);
    DVE forms msg = fs * ex. A one-hot matrix P[e, n] = (dst_rel[e] == n)
    turns the segment softmax sums into PE matmuls accumulating
    [num | den] = P.T @ [msg | ex] in PSUM per window.
  - Window tail: out = num / max(den, eps) + feat_win @ W_res.T.
"""
import sys
import numpy as np

sys.path.insert(0, "/opt/trn_rl_repo")

import concourse.bass as bass  # noqa: E402
import concourse.bacc as bacc  # noqa: E402
import concourse.tile as tile  # noqa: E402
from concourse import mybir  # noqa: E402
from concourse.bass_utils import run_bass_kernel_spmd  # noqa: E402

IN_FEATS = 128
N_HEADS = 4
OUT_FEATS = 32
HF = N_HEADS * OUT_FEATS  # 128
SLOPE = 0.2
P = 128
GRP = 4                              # tiles per group (512 edges)
NSPLIT = 4                           # parts for the big edge-feature inputs
ACT_LRELU = False                    # ACT Lrelu approximates on HW; keep DVE


def _set_sizes(n_nodes=50000, m=8):
    """(Re)compute derived sizing globals. Small sizes used by sim tests."""
    global N_NODES, M, NLOC, WIN, OUT_ROWS
    N_NODES = n_nodes
    M = m                             # cores
    NLOC = N_NODES // M               # dst nodes per core
    WIN = (NLOC + P - 1) // P         # windows per core
    OUT_ROWS = WIN * P


_set_sizes()

f16 = mybir.dt.float16
f32 = mybir.dt.float32

_prog_cache = {}


def _build_program(K: int, n_groups: int):
    """Build the SPMD Bass program for K tiles/window, n_groups groups."""
    nc = bacc.Bacc("TRN2", debug=False, num_devices=M)

    n_tiles = WIN * K
    n_slots = GRP * n_groups * P      # edge slots incl. trailing dummies

    # big edge-feature inputs split into parts: single >28MB inputs crash
    # the surrounding graph compiler's DMA tiling pass
    gpp = (n_groups + NSPLIT - 1) // NSPLIT   # groups per part
    part_rows = [2 * GRP * P * min(gpp, n_groups - i * gpp) for i in range(NSPLIT)]
    feat_sede_p = [nc.dram_tensor(f"feat_sede{i}", [part_rows[i], IN_FEATS], f16,
                                  kind="ExternalInput") for i in range(NSPLIT)]
    feat_win = nc.dram_tensor("feat_win", [OUT_ROWS, IN_FEATS], f16, kind="ExternalInput")
    wsrcT = nc.dram_tensor("wsrcT", [IN_FEATS, HF], f16, kind="ExternalInput")
    wdstT = nc.dram_tensor("wdstT", [IN_FEATS, HF], f16, kind="ExternalInput")
    wresT = nc.dram_tensor("wresT", [IN_FEATS, HF], f16, kind="ExternalInput")
    attn_rep = nc.dram_tensor("attn_rep", [P, GRP * HF], f16, kind="ExternalInput")
    iota_row = nc.dram_tensor("iota_row", [P, GRP * P], f16, kind="ExternalInput")
    drel_tab = nc.dram_tensor("drel_tab", [P, n_groups * GRP], f16, kind="ExternalInput")
    out_d = nc.dram_tensor("out_d", [OUT_ROWS, HF], f32, kind="ExternalOutput")

    with tile.TileContext(nc) as tc:
        with tc.tile_pool(name="const", bufs=1) as cpool:
            ws = cpool.tile([IN_FEATS, HF], f16, tag="w0")
            wd = cpool.tile([IN_FEATS, HF], f16, tag="w1")
            wr = cpool.tile([IN_FEATS, HF], f16, tag="w2")
            at = cpool.tile([P, GRP * HF], f16, tag="attn")
            io = cpool.tile([P, GRP * P], f16, tag="iota")
            idn = cpool.tile([P, P], f16, tag="idn")
            nc.sync.dma_start(ws[:], wsrcT[:])
            nc.sync.dma_start(wd[:], wdstT[:])
            nc.sync.dma_start(wr[:], wresT[:])
            nc.sync.dma_start(at[:], attn_rep[:])
            nc.sync.dma_start(io[:], iota_row[:])
            from concourse.masks import make_identity
            make_identity(nc, idn[:])

            drel_sb = cpool.tile([P, n_groups * GRP], f16, tag="drelsb")
            nc.sync.dma_start(drel_sb[:], drel_tab[:])
            fwin_sb = cpool.tile([P, WIN, IN_FEATS], f16, tag="fwin")
            nc.sync.dma_start(
                fwin_sb[:], feat_win[:].rearrange("(w p) d -> p w d", p=P))
            with tc.tile_pool(name="pb", bufs=4) as pb, \
                 tc.tile_pool(name="ps_fs", bufs=2, space="PSUM") as ps_fs, \
                 tc.tile_pool(name="ps_x", bufs=2, space="PSUM") as ps_x, \
                 tc.tile_pool(name="ps_acc", bufs=2, space="PSUM") as ps_acc, \
                 tc.tile_pool(name="ps_res", bufs=1, space="PSUM") as ps_res:

                acc = None
                for g in range(n_groups):
                    pi, go = divmod(g, gpp)
                    sdT = pb.tile([P, 2 * GRP * P], f16, tag="sdT")
                    nc.sync.dma_start_transpose(
                        sdT[:],
                        feat_sede_p[pi][2 * GRP * P * go:2 * GRP * P * (go + 1), :])
                    seT = sdT[:, 0:GRP * P]
                    deT = sdT[:, GRP * P:2 * GRP * P]
                    dr_t = drel_sb[:, g * GRP:(g + 1) * GRP]

                    fs_ps = ps_fs.tile([P, GRP, HF], f32, tag="fs")
                    x_ps = ps_x.tile([P, GRP, HF], f32, tag="x")
                    for t in range(GRP):
                        sl = slice(t * P, (t + 1) * P)
                        nc.tensor.matmul(fs_ps[:, t, :], lhsT=seT[:, sl],
                                         rhs=ws[:], start=True, stop=True)
                        nc.tensor.matmul(x_ps[:, t, :], lhsT=seT[:, sl],
                                         rhs=ws[:], start=True, stop=False)
                        nc.tensor.matmul(x_ps[:, t, :], lhsT=deT[:, sl],
                                         rhs=wd[:], start=False, stop=True)

                    # z = lrelu(x)
                    z = pb.tile([P, GRP, HF], f16, tag="z")
                    if ACT_LRELU:
                        nc.scalar.activation(
                            out=z[:], in_=x_ps[:],
                            func=mybir.ActivationFunctionType.Lrelu, alpha=SLOPE)
                    else:
                        xsb = pb.tile([P, GRP, HF], f16, tag="xsb")
                        nc.scalar.activation(
                            out=xsb[:], in_=x_ps[:],
                            func=mybir.ActivationFunctionType.Copy)
                        nc.vector.scalar_tensor_tensor(
                            out=z[:], in0=xsb[:], scalar=SLOPE, in1=xsb[:],
                            op0=mybir.AluOpType.mult, op1=mybir.AluOpType.max)
                    # score = sum_f attn * z (per head), ex = exp(score)
                    zm = pb.tile([P, GRP, HF], f16, tag="zm")
                    nc.vector.tensor_tensor(
                        out=zm[:], in0=z[:],
                        in1=at[:].rearrange("p (t d) -> p t d", t=GRP),
                        op=mybir.AluOpType.mult)
                    s = pb.tile([P, GRP * N_HEADS], f32, tag="s")
                    nc.vector.tensor_reduce(
                        out=s[:],
                        in_=zm[:].rearrange("p t (h f) -> p (t h) f", h=N_HEADS),
                        axis=mybir.AxisListType.X, op=mybir.AluOpType.add)
                    ex = pb.tile([P, GRP * N_HEADS], f32, tag="ex")
                    nc.scalar.activation(out=ex[:], in_=s[:],
                                         func=mybir.ActivationFunctionType.Exp)
                    # msgex = [fs * ex | ex]
                    msgex = pb.tile([P, GRP, HF + N_HEADS], f16, tag="msgex")
                    nc.vector.tensor_tensor(
                        out=msgex[:, :, 0:HF].rearrange(
                            "p t (h f) -> p t h f", h=N_HEADS),
                        in0=fs_ps[:].rearrange("p t (h f) -> p t h f", h=N_HEADS),
                        in1=ex[:].rearrange("p (t h) -> p t h", t=GRP)[:, :, :, None]
                            .to_broadcast([P, GRP, N_HEADS, OUT_FEATS]),
                        op=mybir.AluOpType.mult)
                    nc.scalar.activation(
                        out=msgex[:, :, HF:HF + N_HEADS],
                        in_=ex[:].rearrange("p (t h) -> p t h", t=GRP),
                        func=mybir.ActivationFunctionType.Copy)
                    # one-hot P
                    Pt = pb.tile([P, GRP, P], f16, tag="Pt")
                    nc.vector.tensor_tensor(
                        out=Pt[:],
                        in0=io[:].rearrange("p (t d) -> p t d", t=GRP),
                        in1=dr_t[:, :, None].to_broadcast([P, GRP, P]),
                        op=mybir.AluOpType.is_equal)

                    n_slots_t = GRP * n_groups
                    for t in range(GRP):
                        tau = GRP * g + t
                        # trailing dummy tiles (zero one-hot rows) fold into
                        # the last window's accumulation group
                        w = min(tau // K, WIN - 1)
                        k = tau - w * K
                        last_k = (K - 1) if w < WIN - 1 else (n_slots_t - 1 - w * K)
                        if True:
                            if k == 0:
                                acc = ps_acc.tile([P, HF + N_HEADS], f32, tag="acc")
                            nc.tensor.matmul(acc[:], lhsT=Pt[:, t, :],
                                             rhs=msgex[:, t, :],
                                             start=(k == 0), stop=(k == last_k))
                            if k == last_k:
                                # window tail: residual projection + normalize
                                fwT_ps = ps_res.tile([P, P], f16, tag="fwT")
                                nc.tensor.transpose(out=fwT_ps[:],
                                                    in_=fwin_sb[:, w, :],
                                                    identity=idn[:])
                                fwT = pb.tile([P, P], f16, tag="fwTs")
                                nc.scalar.activation(
                                    out=fwT[:], in_=fwT_ps[:],
                                    func=mybir.ActivationFunctionType.Copy)
                                res_ps = ps_res.tile([P, HF], f32, tag="res")
                                nc.tensor.matmul(res_ps[:], lhsT=fwT[:],
                                                 rhs=wr[:], start=True, stop=True)
                                res_sb = pb.tile([P, HF], f32, tag="resb")
                                nc.scalar.activation(
                                    out=res_sb[:], in_=res_ps[:],
                                    func=mybir.ActivationFunctionType.Copy)
                                den = pb.tile([P, N_HEADS], f32, tag="den")
                                nc.vector.tensor_scalar(
                                    out=den[:], in0=acc[:, HF:HF + N_HEADS],
                                    scalar1=1e-30, scalar2=None,
                                    op0=mybir.AluOpType.max)
                                rec = pb.tile([P, N_HEADS], f32, tag="rec")
                                nc.vector.reciprocal(out=rec[:], in_=den[:])
                                osb = pb.tile([P, HF], f32, tag="osb")
                                for h in range(N_HEADS):
                                    sl = slice(h * OUT_FEATS, (h + 1) * OUT_FEATS)
                                    nc.vector.scalar_tensor_tensor(
                                        out=osb[:, sl], in0=acc[:, sl],
                                        scalar=rec[:, h:h + 1], in1=res_sb[:, sl],
                                        op0=mybir.AluOpType.mult,
                                        op1=mybir.AluOpType.add)
                                nc.sync.dma_start(
                                    out_d[w * P:(w + 1) * P, :], osb[:])

    nc.compile()
    return nc


def _preprocess(feat, W_src, b_src, W_dst, b_dst, attn_e, W_res, b_res, src, dst):
    """Host-side sharding: sort edges by dst, build per-core inputs."""
    feat = np.asarray(feat, dtype=np.float32)
    b_src = np.asarray(b_src, np.float32)
    b_dst = np.asarray(b_dst, np.float32)
    b_res = np.asarray(b_res, np.float32)
    assert not (b_src.any() or b_dst.any() or b_res.any()), \
        "nonzero biases not supported by this kernel build"
    src = np.asarray(src, dtype=np.int64)
    dst = np.asarray(dst, dtype=np.int64)

    order = np.argsort(dst, kind="stable")
    src_s = src[order]
    dst_s = dst[order]

    dev_bounds = np.searchsorted(dst_s, np.arange(M + 1) * NLOC)
    per_dev = []
    K = 1
    for d in range(M):
        e0, e1 = dev_bounds[d], dev_bounds[d + 1]
        sd = src_s[e0:e1]
        dd = dst_s[e0:e1] - d * NLOC
        w = dd // P
        starts = np.searchsorted(dd, np.arange(WIN) * P)
        r = np.arange(len(dd)) - starts[w]
        counts = np.diff(np.searchsorted(dd, np.arange(0, WIN * P + P, P)))
        if len(dd):
            K = max(K, int((counts.max() + P - 1) // P))
        per_dev.append((sd, dd, w, r))

    n_tiles = WIN * K
    n_groups = (n_tiles + GRP - 1) // GRP
    n_slots = GRP * n_groups * P

    feat16 = feat.astype(np.float16)
    # slot -> src node id / dst node id (0 for pad slots: their P row is all
    # zero so the scatter multiplies them by zero)
    se_ids = np.zeros((M, n_slots), dtype=np.int64)
    de_ids = np.zeros((M, n_slots), dtype=np.int64)
    drel_all = np.full((M, n_groups, P, GRP), -1.0, dtype=np.float16)

    for d in range(M):
        sd, dd, w, r = per_dev[d]
        if not len(dd):
            continue
        tau = w * K + r // P
        p = r % P
        slot = tau * P + p            # tile-major, partition-minor
        se_ids[d, slot] = sd
        de_ids[d, slot] = dd + d * NLOC
        g = tau // GRP
        t = tau % GRP
        drel_all[d, g, p, t] = (dd - w * P).astype(np.float16)

    feat_wins = []
    for d in range(M):
        fwin = np.zeros((OUT_ROWS, IN_FEATS), dtype=np.float16)
        fwin[:NLOC] = feat16[d * NLOC:(d + 1) * NLOC]
        feat_wins.append(fwin)

    cst = {
        "wsrcT": np.ascontiguousarray(np.asarray(W_src, np.float32).T).astype(np.float16),
        "wdstT": np.ascontiguousarray(np.asarray(W_dst, np.float32).T).astype(np.float16),
        "wresT": np.ascontiguousarray(np.asarray(W_res, np.float32).T).astype(np.float16),
        "attn_rep": np.tile(np.asarray(attn_e, np.float32).reshape(1, HF),
                            (P, GRP)).astype(np.float16),
        "iota_row": np.tile(np.arange(P, dtype=np.float16)[None, :], (P, GRP)),
    }

    gpp = (n_groups + NSPLIT - 1) // NSPLIT
    in_maps = []
    for d in range(M):
        m = dict(cst)
        # interleave per group: [se rows | de rows] in one block
        fse = feat16[se_ids[d]].reshape(n_groups, GRP * P, IN_FEATS)
        fde = feat16[de_ids[d]].reshape(n_groups, GRP * P, IN_FEATS)
        sede = np.concatenate([fse, fde], axis=1)  # [ng, 2*GRP*P, D]
        for i in range(NSPLIT):
            g0 = gpp * i
            g1 = min(gpp * (i + 1), n_groups)
            m[f"feat_sede{i}"] = sede[g0:g1].reshape(-1, IN_FEATS)
        m["feat_win"] = feat_wins[d]
        m["drel_tab"] = np.ascontiguousarray(drel_all[d].transpose(1, 0, 2).reshape(P, -1))
        in_maps.append(m)
    return K, n_groups, in_maps


def kernel(feat, W_src, b_src, W_dst, b_dst, attn_e, W_res, b_res, src, dst,
           _trace=False, _trace_kwargs=None):
    K, n_groups, in_maps = _preprocess(feat, W_src, b_src, W_dst, b_dst,
                                       attn_e, W_res, b_res, src, dst)
    key = (K, n_groups)
    if key not in _prog_cache:
        _prog_cache[key] = _build_program(K, n_groups)
    nc = _prog_cache[key]

    kw = {}
    if _trace:
        kw = dict(trace=True, trace_kwargs=_trace_kwargs or {})
    res = run_bass_kernel_spmd(nc, in_maps, core_ids=list(range(M)), **kw)
    outs = [res.results[d]["out_d"][:NLOC] for d in range(M)]
    full = np.concatenate(outs, axis=0).reshape(N_NODES, N_HEADS, OUT_FEATS)
    kernel._last_results = res
    kernel._last_cfg = (K, n_groups)
    return full


# revision 28
# speedup vs baseline: 1.0609x; 1.0609x over previous
"""GATv2 layer on 8 Trainium2 NeuronCores (Bass/Tile).

Sharding (edge-parallel by destination range, per the problem's hint: each
device holds its edge shard plus gathered src/dst node features):
  - Device d owns destination nodes [d*6250, (d+1)*6250).
  - Host sorts edges by dst, routes each edge to the device owning its dst,
    and materializes the gathered node features feat[src], feat[dst] per edge
    slot (fp16) — these are the per-device inputs, so the device streams
    contiguous data instead of doing random-access gathers.
  - Per device, dst nodes are grouped into 49 windows of 128 nodes. Each
    window's incoming edges are padded to K tiles of 128 edges (K = global
    max, data-driven) so every core runs the identical SPMD program.
  - Per 512-edge group, transpose-DMA loads stream featT tiles; PE matmuls
    compute fs = feat_se @ W_src.T and x = fs + fd directly in PSUM;
    ACT applies LeakyReLU; DVE forms score = sum_f attn*z per head;
    ACT exponentiates (no max subtraction: scores are O(1) by construction
    and softmax is shift-invariant, matching the reference exactly);
    DVE forms msg = fs * ex. A one-hot matrix P[e, n] = (dst_rel[e] == n)
    turns the segment softmax sums into PE matmuls accumulating
    [num | den] = P.T @ [msg | ex] in PSUM per window.
  - Window tail: out = num / max(den, eps) + feat_win @ W_res.T.
"""
import sys
import numpy as np

sys.path.insert(0, "/opt/trn_rl_repo")

import concourse.bass as bass  # noqa: E402
import concourse.bacc as bacc  # noqa: E402
import concourse.tile as tile  # noqa: E402
from concourse import mybir  # noqa: E402
from concourse.bass_utils import run_bass_kernel_spmd  # noqa: E402

IN_FEATS = 128
N_HEADS = 4
OUT_FEATS = 32
HF = N_HEADS * OUT_FEATS  # 128
SLOPE = 0.2
P = 128
GRP = 4                              # tiles per group (512 edges)
NSPLIT = 4                           # parts for the big edge-feature inputs
ACT_LRELU = False                    # ACT Lrelu approximates on HW; keep DVE


def _set_sizes(n_nodes=50000, m=8):
    """(Re)compute derived sizing globals. Small sizes used by sim tests."""
    global N_NODES, M, NLOC, WIN, OUT_ROWS
    N_NODES = n_nodes
    M = m                             # cores
    NLOC = N_NODES // M               # dst nodes per core
    WIN = (NLOC + P - 1) // P         # windows per core
    OUT_ROWS = WIN * P


_set_sizes()

f16 = mybir.dt.float16
f32 = mybir.dt.float32

_prog_cache = {}


def _build_program(K: int, n_groups: int):
    """Build the SPMD Bass program for K tiles/window, n_groups groups."""
    nc = bacc.Bacc("TRN2", debug=False, num_devices=M)

    n_tiles = WIN * K
    n_slots = GRP * n_groups * P      # edge slots incl. trailing dummies

    # big edge-feature inputs split into parts: single >28MB inputs crash
    # the surrounding graph compiler's DMA tiling pass
    gpp = (n_groups + NSPLIT - 1) // NSPLIT   # groups per part
    part_rows = [2 * GRP * P * min(gpp, n_groups - i * gpp) for i in range(NSPLIT)]
    feat_sede_p = [nc.dram_tensor(f"feat_sede{i}", [part_rows[i], IN_FEATS], f16,
                                  kind="ExternalInput") for i in range(NSPLIT)]
    feat_win = nc.dram_tensor("feat_win", [OUT_ROWS, IN_FEATS], f16, kind="ExternalInput")
    wsrcT = nc.dram_tensor("wsrcT", [IN_FEATS, HF], f16, kind="ExternalInput")
    wys = nc.dram_tensor("wys", [IN_FEATS, HF + N_HEADS], f16, kind="ExternalInput")
    wyd = nc.dram_tensor("wyd", [IN_FEATS, HF + N_HEADS], f16, kind="ExternalInput")
    wresT = nc.dram_tensor("wresT", [IN_FEATS, HF], f16, kind="ExternalInput")
    sign04 = nc.dram_tensor("sign04", [P, GRP * HF], f16, kind="ExternalInput")
    iota_row = nc.dram_tensor("iota_row", [P, GRP * P], f16, kind="ExternalInput")
    drel_tab = nc.dram_tensor("drel_tab", [P, n_groups * GRP], f16, kind="ExternalInput")
    out_d = nc.dram_tensor("out_d", [OUT_ROWS, HF], f32, kind="ExternalOutput")

    with tile.TileContext(nc) as tc:
        with tc.tile_pool(name="const", bufs=1) as cpool:
            ws = cpool.tile([IN_FEATS, HF], f16, tag="w0")
            wy1 = cpool.tile([IN_FEATS, HF + N_HEADS], f16, tag="wy1")
            wy2 = cpool.tile([IN_FEATS, HF + N_HEADS], f16, tag="wy2")
            wr = cpool.tile([IN_FEATS, HF], f16, tag="w2")
            at = cpool.tile([P, GRP * HF], f16, tag="attn")
            io = cpool.tile([P, GRP * P], f16, tag="iota")
            idn = cpool.tile([P, P], f16, tag="idn")
            nc.sync.dma_start(ws[:], wsrcT[:])
            nc.sync.dma_start(wy1[:], wys[:])
            nc.sync.dma_start(wy2[:], wyd[:])
            nc.sync.dma_start(wr[:], wresT[:])
            nc.sync.dma_start(at[:], sign04[:])
            nc.sync.dma_start(io[:], iota_row[:])
            from concourse.masks import make_identity
            make_identity(nc, idn[:])

            drel_sb = cpool.tile([P, n_groups * GRP], f16, tag="drelsb")
            nc.sync.dma_start(drel_sb[:], drel_tab[:])
            fwin_sb = cpool.tile([P, WIN, IN_FEATS], f16, tag="fwin")
            nc.sync.dma_start(
                fwin_sb[:], feat_win[:].rearrange("(w p) d -> p w d", p=P))
            with tc.tile_pool(name="pb", bufs=4) as pb, \
                 tc.tile_pool(name="ps_fs", bufs=2, space="PSUM") as ps_fs, \
                 tc.tile_pool(name="ps_x", bufs=1, space="PSUM") as ps_x, \
                 tc.tile_pool(name="ps_acc", bufs=2, space="PSUM") as ps_acc, \
                 tc.tile_pool(name="ps_res", bufs=1, space="PSUM") as ps_res:

                acc = None
                for g in range(n_groups):
                    pi, go = divmod(g, gpp)
                    sdT = pb.tile([P, 2 * GRP * P], f16, tag="sdT")
                    nc.sync.dma_start_transpose(
                        sdT[:],
                        feat_sede_p[pi][2 * GRP * P * go:2 * GRP * P * (go + 1), :])
                    seT = sdT[:, 0:GRP * P]
                    deT = sdT[:, GRP * P:2 * GRP * P]
                    dr_t = drel_sb[:, g * GRP:(g + 1) * GRP]

                    fs_ps = ps_fs.tile([P, GRP, HF], f32, tag="fs")
                    y_ps = ps_x.tile([P, GRP, 256], f32, tag="x")
                    for t in range(GRP):
                        sl = slice(t * P, (t + 1) * P)
                        nc.tensor.matmul(fs_ps[:, t, :], lhsT=seT[:, sl],
                                         rhs=ws[:], start=True, stop=True)
                        nc.tensor.matmul(y_ps[:, t, 0:HF + N_HEADS],
                                         lhsT=seT[:, sl],
                                         rhs=wy1[:], start=True, stop=False)
                        nc.tensor.matmul(y_ps[:, t, 0:HF + N_HEADS],
                                         lhsT=deT[:, sl],
                                         rhs=wy2[:], start=False, stop=True)

                    # score = 0.6*sum attn*x + 0.4*sum sign(attn)*|attn*x|
                    ay = pb.tile([P, GRP, HF], f16, tag="ay")
                    nc.scalar.activation(
                        out=ay[:], in_=y_ps[:, :, 0:HF],
                        func=mybir.ActivationFunctionType.Abs)
                    zm = pb.tile([P, GRP, HF], f16, tag="zm")
                    nc.vector.tensor_tensor(
                        out=zm[:], in0=ay[:],
                        in1=at[:].rearrange("p (t d) -> p t d", t=GRP),
                        op=mybir.AluOpType.mult)
                    r = pb.tile([P, GRP * N_HEADS], f32, tag="r")
                    nc.vector.tensor_reduce(
                        out=r[:],
                        in_=zm[:].rearrange("p t (h f) -> p (t h) f", h=N_HEADS),
                        axis=mybir.AxisListType.X, op=mybir.AluOpType.add)
                    s = pb.tile([P, GRP * N_HEADS], f32, tag="s")
                    nc.vector.tensor_tensor(
                        out=s[:].rearrange("p (t h) -> p t h", t=GRP),
                        in0=r[:].rearrange("p (t h) -> p t h", t=GRP),
                        in1=y_ps[:, :, HF:HF + N_HEADS],
                        op=mybir.AluOpType.add)
                    ex = pb.tile([P, GRP * N_HEADS], f32, tag="ex")
                    nc.scalar.activation(out=ex[:], in_=s[:],
                                         func=mybir.ActivationFunctionType.Exp)
                    # msgex = [fs * ex | ex]
                    msgex = pb.tile([P, GRP, HF + N_HEADS], f16, tag="msgex")
                    nc.vector.tensor_tensor(
                        out=msgex[:, :, 0:HF].rearrange(
                            "p t (h f) -> p t h f", h=N_HEADS),
                        in0=fs_ps[:].rearrange("p t (h f) -> p t h f", h=N_HEADS),
                        in1=ex[:].rearrange("p (t h) -> p t h", t=GRP)[:, :, :, None]
                            .to_broadcast([P, GRP, N_HEADS, OUT_FEATS]),
                        op=mybir.AluOpType.mult)
                    nc.scalar.activation(
                        out=msgex[:, :, HF:HF + N_HEADS],
                        in_=ex[:].rearrange("p (t h) -> p t h", t=GRP),
                        func=mybir.ActivationFunctionType.Copy)
                    # one-hot P
                    Pt = pb.tile([P, GRP, P], f16, tag="Pt")
                    nc.vector.tensor_tensor(
                        out=Pt[:],
                        in0=io[:].rearrange("p (t d) -> p t d", t=GRP),
                        in1=dr_t[:, :, None].to_broadcast([P, GRP, P]),
                        op=mybir.AluOpType.is_equal)

                    n_slots_t = GRP * n_groups
                    for t in range(GRP):
                        tau = GRP * g + t
                        # trailing dummy tiles (zero one-hot rows) fold into
                        # the last window's accumulation group
                        w = min(tau // K, WIN - 1)
                        k = tau - w * K
                        last_k = (K - 1) if w < WIN - 1 else (n_slots_t - 1 - w * K)
                        if True:
                            if k == 0:
                                acc = ps_acc.tile([P, HF + N_HEADS], f32, tag="acc")
                            nc.tensor.matmul(acc[:], lhsT=Pt[:, t, :],
                                             rhs=msgex[:, t, :],
                                             start=(k == 0), stop=(k == last_k))
                            if k == last_k:
                                # window tail: residual projection + normalize
                                fwT_ps = ps_res.tile([P, P], f16, tag="fwT")
                                nc.tensor.transpose(out=fwT_ps[:],
                                                    in_=fwin_sb[:, w, :],
                                                    identity=idn[:])
                                fwT = pb.tile([P, P], f16, tag="fwTs")
                                nc.scalar.activation(
                                    out=fwT[:], in_=fwT_ps[:],
                                    func=mybir.ActivationFunctionType.Copy)
                                res_ps = ps_res.tile([P, HF], f32, tag="res")
                                nc.tensor.matmul(res_ps[:], lhsT=fwT[:],
                                                 rhs=wr[:], start=True, stop=True)
                                res_sb = pb.tile([P, HF], f32, tag="resb")
                                nc.scalar.activation(
                                    out=res_sb[:], in_=res_ps[:],
                                    func=mybir.ActivationFunctionType.Copy)
                                den = pb.tile([P, N_HEADS], f32, tag="den")
                                nc.vector.tensor_scalar(
                                    out=den[:], in0=acc[:, HF:HF + N_HEADS],
                                    scalar1=1e-30, scalar2=None,
                                    op0=mybir.AluOpType.max)
                                rec = pb.tile([P, N_HEADS], f32, tag="rec")
                                nc.vector.reciprocal(out=rec[:], in_=den[:])
                                osb = pb.tile([P, HF], f32, tag="osb")
                                for h in range(N_HEADS):
                                    sl = slice(h * OUT_FEATS, (h + 1) * OUT_FEATS)
                                    nc.vector.scalar_tensor_tensor(
                                        out=osb[:, sl], in0=acc[:, sl],
                                        scalar=rec[:, h:h + 1], in1=res_sb[:, sl],
                                        op0=mybir.AluOpType.mult,
                                        op1=mybir.AluOpType.add)
                                nc.sync.dma_start(
                                    out_d[w * P:(w + 1) * P, :], osb[:])

    nc.compile()
    return nc


def _preprocess(feat, W_src, b_src, W_dst, b_dst, attn_e, W_res, b_res, src, dst):
    """Host-side sharding: sort edges by dst, build per-core inputs."""
    feat = np.asarray(feat, dtype=np.float32)
    b_src = np.asarray(b_src, np.float32)
    b_dst = np.asarray(b_dst, np.float32)
    b_res = np.asarray(b_res, np.float32)
    assert not (b_src.any() or b_dst.any() or b_res.any()), \
        "nonzero biases not supported by this kernel build"
    src = np.asarray(src, dtype=np.int64)
    dst = np.asarray(dst, dtype=np.int64)

    order = np.argsort(dst, kind="stable")
    src_s = src[order]
    dst_s = dst[order]

    dev_bounds = np.searchsorted(dst_s, np.arange(M + 1) * NLOC)
    per_dev = []
    K = 1
    for d in range(M):
        e0, e1 = dev_bounds[d], dev_bounds[d + 1]
        sd = src_s[e0:e1]
        dd = dst_s[e0:e1] - d * NLOC
        w = dd // P
        starts = np.searchsorted(dd, np.arange(WIN) * P)
        r = np.arange(len(dd)) - starts[w]
        counts = np.diff(np.searchsorted(dd, np.arange(0, WIN * P + P, P)))
        if len(dd):
            K = max(K, int((counts.max() + P - 1) // P))
        per_dev.append((sd, dd, w, r))

    n_tiles = WIN * K
    n_groups = (n_tiles + GRP - 1) // GRP
    n_slots = GRP * n_groups * P

    feat16 = feat.astype(np.float16)
    # slot -> src node id / dst node id (0 for pad slots: their P row is all
    # zero so the scatter multiplies them by zero)
    se_ids = np.zeros((M, n_slots), dtype=np.int64)
    de_ids = np.zeros((M, n_slots), dtype=np.int64)
    drel_all = np.full((M, n_groups, P, GRP), -1.0, dtype=np.float16)

    for d in range(M):
        sd, dd, w, r = per_dev[d]
        if not len(dd):
            continue
        tau = w * K + r // P
        p = r % P
        slot = tau * P + p            # tile-major, partition-minor
        se_ids[d, slot] = sd
        de_ids[d, slot] = dd + d * NLOC
        g = tau // GRP
        t = tau % GRP
        drel_all[d, g, p, t] = (dd - w * P).astype(np.float16)

    feat_wins = []
    for d in range(M):
        fwin = np.zeros((OUT_ROWS, IN_FEATS), dtype=np.float16)
        fwin[:NLOC] = feat16[d * NLOC:(d + 1) * NLOC]
        feat_wins.append(fwin)

    attn_f = np.asarray(attn_e, np.float32).reshape(HF)
    attn_hf = attn_f.reshape(N_HEADS, OUT_FEATS)

    def aug(WT):
        y = WT * attn_f[None, :]
        u = 0.6 * np.einsum("ahf,hf->ah", WT.reshape(IN_FEATS, N_HEADS,
                                                     OUT_FEATS), attn_hf)
        return np.concatenate([y, u], axis=1).astype(np.float16)

    WsT = np.ascontiguousarray(np.asarray(W_src, np.float32).T)
    WdT = np.ascontiguousarray(np.asarray(W_dst, np.float32).T)
    cst = {
        "wsrcT": WsT.astype(np.float16),
        "wys": aug(WsT),
        "wyd": aug(WdT),
        "wresT": np.ascontiguousarray(np.asarray(W_res, np.float32).T).astype(np.float16),
        "sign04": np.tile((0.4 * np.sign(attn_f)).astype(np.float16)[None, :],
                          (P, GRP)),
        "iota_row": np.tile(np.arange(P, dtype=np.float16)[None, :], (P, GRP)),
    }

    gpp = (n_groups + NSPLIT - 1) // NSPLIT
    in_maps = []
    for d in range(M):
        m = dict(cst)
        # interleave per group: [se rows | de rows] in one block
        fse = feat16[se_ids[d]].reshape(n_groups, GRP * P, IN_FEATS)
        fde = feat16[de_ids[d]].reshape(n_groups, GRP * P, IN_FEATS)
        sede = np.concatenate([fse, fde], axis=1)  # [ng, 2*GRP*P, D]
        for i in range(NSPLIT):
            g0 = gpp * i
            g1 = min(gpp * (i + 1), n_groups)
            m[f"feat_sede{i}"] = sede[g0:g1].reshape(-1, IN_FEATS)
        m["feat_win"] = feat_wins[d]
        m["drel_tab"] = np.ascontiguousarray(drel_all[d].transpose(1, 0, 2).reshape(P, -1))
        in_maps.append(m)
    return K, n_groups, in_maps


def kernel(feat, W_src, b_src, W_dst, b_dst, attn_e, W_res, b_res, src, dst,
           _trace=False, _trace_kwargs=None):
    K, n_groups, in_maps = _preprocess(feat, W_src, b_src, W_dst, b_dst,
                                       attn_e, W_res, b_res, src, dst)
    key = (K, n_groups)
    if key not in _prog_cache:
        _prog_cache[key] = _build_program(K, n_groups)
    nc = _prog_cache[key]

    kw = {}
    if _trace:
        kw = dict(trace=True, trace_kwargs=_trace_kwargs or {})
    res = run_bass_kernel_spmd(nc, in_maps, core_ids=list(range(M)), **kw)
    outs = [res.results[d]["out_d"][:NLOC] for d in range(M)]
    full = np.concatenate(outs, axis=0).reshape(N_NODES, N_HEADS, OUT_FEATS)
    kernel._last_results = res
    kernel._last_cfg = (K, n_groups)
    return full


# revision 29
# speedup vs baseline: 1.1467x; 1.0809x over previous
"""GATv2 layer on 8 Trainium2 NeuronCores (Bass/Tile).

Sharding (edge-parallel by destination range, per the problem's hint: each
device holds its edge shard plus gathered src/dst node features):
  - Device d owns destination nodes [d*6250, (d+1)*6250).
  - Host sorts edges by dst, routes each edge to the device owning its dst,
    and materializes the gathered node features feat[src], feat[dst] per edge
    slot (fp16) — these are the per-device inputs, so the device streams
    contiguous data instead of doing random-access gathers.
  - Per device, dst nodes are grouped into 49 windows of 128 nodes. Each
    window's incoming edges are padded to K tiles of 128 edges (K = global
    max, data-driven) so every core runs the identical SPMD program.
  - Per 512-edge group, transpose-DMA loads stream featT tiles; PE matmuls
    compute fs = feat_se @ W_src.T and x = fs + fd directly in PSUM;
    ACT applies LeakyReLU; DVE forms score = sum_f attn*z per head;
    ACT exponentiates (no max subtraction: scores are O(1) by construction
    and softmax is shift-invariant, matching the reference exactly);
    DVE forms msg = fs * ex. A one-hot matrix P[e, n] = (dst_rel[e] == n)
    turns the segment softmax sums into PE matmuls accumulating
    [num | den] = P.T @ [msg | ex] in PSUM per window.
  - Window tail: out = num / max(den, eps) + feat_win @ W_res.T.
"""
import sys
import numpy as np

sys.path.insert(0, "/opt/trn_rl_repo")

import concourse.bass as bass  # noqa: E402
import concourse.bacc as bacc  # noqa: E402
import concourse.tile as tile  # noqa: E402
from concourse import mybir  # noqa: E402
from concourse.bass_utils import run_bass_kernel_spmd  # noqa: E402

IN_FEATS = 128
N_HEADS = 4
OUT_FEATS = 32
HF = N_HEADS * OUT_FEATS  # 128
SLOPE = 0.2
P = 128
GRP = 4                              # tiles per group (512 edges)
NSPLIT = 4                           # parts for the big edge-feature inputs
ACT_LRELU = False                    # ACT Lrelu approximates on HW; keep DVE


def _set_sizes(n_nodes=50000, m=8):
    """(Re)compute derived sizing globals. Small sizes used by sim tests."""
    global N_NODES, M, NLOC, WIN, OUT_ROWS
    N_NODES = n_nodes
    M = m                             # cores
    NLOC = N_NODES // M               # dst nodes per core
    WIN = (NLOC + P - 1) // P         # windows per core
    OUT_ROWS = WIN * P


_set_sizes()

f16 = mybir.dt.float16
f32 = mybir.dt.float32

_prog_cache = {}


def _build_program(K: int, n_groups: int):
    """Build the SPMD Bass program for K tiles/window, n_groups groups."""
    nc = bacc.Bacc("TRN2", debug=False, num_devices=M)

    n_tiles = WIN * K
    n_slots = GRP * n_groups * P      # edge slots incl. trailing dummies

    # big edge-feature inputs split into parts: single >28MB inputs crash
    # the surrounding graph compiler's DMA tiling pass
    gpp = (n_groups + NSPLIT - 1) // NSPLIT   # groups per part
    part_rows = [3 * GRP * P * min(gpp, n_groups - i * gpp) for i in range(NSPLIT)]
    feat_sede_p = [nc.dram_tensor(f"feat_sede{i}", [part_rows[i], IN_FEATS], f16,
                                  kind="ExternalInput") for i in range(NSPLIT)]
    feat_win = nc.dram_tensor("feat_win", [OUT_ROWS, IN_FEATS], f16, kind="ExternalInput")
    wsrcT = nc.dram_tensor("wsrcT", [IN_FEATS, HF], f16, kind="ExternalInput")
    wys = nc.dram_tensor("wys", [IN_FEATS, HF + N_HEADS], f16, kind="ExternalInput")
    wyd = nc.dram_tensor("wyd", [IN_FEATS, HF + N_HEADS], f16, kind="ExternalInput")
    wresT = nc.dram_tensor("wresT", [IN_FEATS, HF], f16, kind="ExternalInput")
    sign04 = nc.dram_tensor("sign04", [P, GRP * HF], f16, kind="ExternalInput")
    out_d = nc.dram_tensor("out_d", [OUT_ROWS, HF], f32, kind="ExternalOutput")

    with tile.TileContext(nc) as tc:
        with tc.tile_pool(name="const", bufs=1) as cpool:
            ws = cpool.tile([IN_FEATS, HF], f16, tag="w0")
            wy1 = cpool.tile([IN_FEATS, HF + N_HEADS], f16, tag="wy1")
            wy2 = cpool.tile([IN_FEATS, HF + N_HEADS], f16, tag="wy2")
            wr = cpool.tile([IN_FEATS, HF], f16, tag="w2")
            at = cpool.tile([P, GRP * HF], f16, tag="attn")
            idn = cpool.tile([P, P], f16, tag="idn")
            nc.sync.dma_start(ws[:], wsrcT[:])
            nc.sync.dma_start(wy1[:], wys[:])
            nc.sync.dma_start(wy2[:], wyd[:])
            nc.sync.dma_start(wr[:], wresT[:])
            nc.sync.dma_start(at[:], sign04[:])
            from concourse.masks import make_identity
            make_identity(nc, idn[:])

            fwin_sb = cpool.tile([P, WIN, IN_FEATS], f16, tag="fwin")
            nc.sync.dma_start(
                fwin_sb[:], feat_win[:].rearrange("(w p) d -> p w d", p=P))
            with tc.tile_pool(name="pb", bufs=4) as pb, \
                 tc.tile_pool(name="ps_fs", bufs=2, space="PSUM") as ps_fs, \
                 tc.tile_pool(name="ps_x", bufs=1, space="PSUM") as ps_x, \
                 tc.tile_pool(name="ps_acc", bufs=2, space="PSUM") as ps_acc, \
                 tc.tile_pool(name="ps_res", bufs=1, space="PSUM") as ps_res:

                acc = None
                for g in range(n_groups):
                    pi, go = divmod(g, gpp)
                    sdT = pb.tile([P, 3 * GRP * P], f16, tag="sdT")
                    nc.sync.dma_start_transpose(
                        sdT[:],
                        feat_sede_p[pi][3 * GRP * P * go:3 * GRP * P * (go + 1), :])
                    seT = sdT[:, 0:GRP * P]
                    deT = sdT[:, GRP * P:2 * GRP * P]
                    Pt = sdT[:, 2 * GRP * P:3 * GRP * P].rearrange(
                        "p (t d) -> p t d", t=GRP)

                    fs_ps = ps_fs.tile([P, GRP, HF], f32, tag="fs")
                    y_ps = ps_x.tile([P, GRP, 256], f32, tag="x")
                    for t in range(GRP):
                        sl = slice(t * P, (t + 1) * P)
                        nc.tensor.matmul(fs_ps[:, t, :], lhsT=seT[:, sl],
                                         rhs=ws[:], start=True, stop=True)
                        nc.tensor.matmul(y_ps[:, t, 0:HF + N_HEADS],
                                         lhsT=seT[:, sl],
                                         rhs=wy1[:], start=True, stop=False)
                        nc.tensor.matmul(y_ps[:, t, 0:HF + N_HEADS],
                                         lhsT=deT[:, sl],
                                         rhs=wy2[:], start=False, stop=True)

                    # score = 0.6*sum attn*x + 0.4*sum sign(attn)*|attn*x|
                    ay = pb.tile([P, GRP, HF], f16, tag="ay")
                    nc.scalar.activation(
                        out=ay[:], in_=y_ps[:, :, 0:HF],
                        func=mybir.ActivationFunctionType.Abs)
                    zm = pb.tile([P, GRP, HF], f16, tag="zm")
                    nc.vector.tensor_tensor(
                        out=zm[:], in0=ay[:],
                        in1=at[:].rearrange("p (t d) -> p t d", t=GRP),
                        op=mybir.AluOpType.mult)
                    r = pb.tile([P, GRP * N_HEADS], f32, tag="r")
                    nc.vector.tensor_reduce(
                        out=r[:],
                        in_=zm[:].rearrange("p t (h f) -> p (t h) f", h=N_HEADS),
                        axis=mybir.AxisListType.X, op=mybir.AluOpType.add)
                    s = pb.tile([P, GRP * N_HEADS], f32, tag="s")
                    nc.vector.tensor_tensor(
                        out=s[:].rearrange("p (t h) -> p t h", t=GRP),
                        in0=r[:].rearrange("p (t h) -> p t h", t=GRP),
                        in1=y_ps[:, :, HF:HF + N_HEADS],
                        op=mybir.AluOpType.add)
                    ex = pb.tile([P, GRP * N_HEADS], f32, tag="ex")
                    nc.scalar.activation(out=ex[:], in_=s[:],
                                         func=mybir.ActivationFunctionType.Exp)
                    # msgex = [fs * ex | ex]
                    msgex = pb.tile([P, GRP, HF + N_HEADS], f16, tag="msgex")
                    nc.vector.tensor_tensor(
                        out=msgex[:, :, 0:HF].rearrange(
                            "p t (h f) -> p t h f", h=N_HEADS),
                        in0=fs_ps[:].rearrange("p t (h f) -> p t h f", h=N_HEADS),
                        in1=ex[:].rearrange("p (t h) -> p t h", t=GRP)[:, :, :, None]
                            .to_broadcast([P, GRP, N_HEADS, OUT_FEATS]),
                        op=mybir.AluOpType.mult)
                    nc.scalar.activation(
                        out=msgex[:, :, HF:HF + N_HEADS],
                        in_=ex[:].rearrange("p (t h) -> p t h", t=GRP),
                        func=mybir.ActivationFunctionType.Copy)

                    n_slots_t = GRP * n_groups
                    for t in range(GRP):
                        tau = GRP * g + t
                        # trailing dummy tiles (zero one-hot rows) fold into
                        # the last window's accumulation group
                        w = min(tau // K, WIN - 1)
                        k = tau - w * K
                        last_k = (K - 1) if w < WIN - 1 else (n_slots_t - 1 - w * K)
                        if True:
                            if k == 0:
                                acc = ps_acc.tile([P, HF + N_HEADS], f32, tag="acc")
                            nc.tensor.matmul(acc[:], lhsT=Pt[:, t, :],
                                             rhs=msgex[:, t, :],
                                             start=(k == 0), stop=(k == last_k))
                            if k == last_k:
                                # window tail: residual projection + normalize
                                fwT_ps = ps_res.tile([P, P], f16, tag="fwT")
                                nc.tensor.transpose(out=fwT_ps[:],
                                                    in_=fwin_sb[:, w, :],
                                                    identity=idn[:])
                                fwT = pb.tile([P, P], f16, tag="fwTs")
                                nc.scalar.activation(
                                    out=fwT[:], in_=fwT_ps[:],
                                    func=mybir.ActivationFunctionType.Copy)
                                res_ps = ps_res.tile([P, HF], f32, tag="res")
                                nc.tensor.matmul(res_ps[:], lhsT=fwT[:],
                                                 rhs=wr[:], start=True, stop=True)
                                res_sb = pb.tile([P, HF], f32, tag="resb")
                                nc.scalar.activation(
                                    out=res_sb[:], in_=res_ps[:],
                                    func=mybir.ActivationFunctionType.Copy)
                                den = pb.tile([P, N_HEADS], f32, tag="den")
                                nc.vector.tensor_scalar(
                                    out=den[:], in0=acc[:, HF:HF + N_HEADS],
                                    scalar1=1e-30, scalar2=None,
                                    op0=mybir.AluOpType.max)
                                rec = pb.tile([P, N_HEADS], f32, tag="rec")
                                nc.vector.reciprocal(out=rec[:], in_=den[:])
                                osb = pb.tile([P, HF], f32, tag="osb")
                                for h in range(N_HEADS):
                                    sl = slice(h * OUT_FEATS, (h + 1) * OUT_FEATS)
                                    nc.vector.scalar_tensor_tensor(
                                        out=osb[:, sl], in0=acc[:, sl],
                                        scalar=rec[:, h:h + 1], in1=res_sb[:, sl],
                                        op0=mybir.AluOpType.mult,
                                        op1=mybir.AluOpType.add)
                                nc.sync.dma_start(
                                    out_d[w * P:(w + 1) * P, :], osb[:])

    nc.compile()
    return nc


def _preprocess(feat, W_src, b_src, W_dst, b_dst, attn_e, W_res, b_res, src, dst):
    """Host-side sharding: sort edges by dst, build per-core inputs."""
    feat = np.asarray(feat, dtype=np.float32)
    b_src = np.asarray(b_src, np.float32)
    b_dst = np.asarray(b_dst, np.float32)
    b_res = np.asarray(b_res, np.float32)
    assert not (b_src.any() or b_dst.any() or b_res.any()), \
        "nonzero biases not supported by this kernel build"
    src = np.asarray(src, dtype=np.int64)
    dst = np.asarray(dst, dtype=np.int64)

    order = np.argsort(dst, kind="stable")
    src_s = src[order]
    dst_s = dst[order]

    dev_bounds = np.searchsorted(dst_s, np.arange(M + 1) * NLOC)
    per_dev = []
    K = 1
    for d in range(M):
        e0, e1 = dev_bounds[d], dev_bounds[d + 1]
        sd = src_s[e0:e1]
        dd = dst_s[e0:e1] - d * NLOC
        w = dd // P
        starts = np.searchsorted(dd, np.arange(WIN) * P)
        r = np.arange(len(dd)) - starts[w]
        counts = np.diff(np.searchsorted(dd, np.arange(0, WIN * P + P, P)))
        if len(dd):
            K = max(K, int((counts.max() + P - 1) // P))
        per_dev.append((sd, dd, w, r))

    n_tiles = WIN * K
    n_groups = (n_tiles + GRP - 1) // GRP
    n_slots = GRP * n_groups * P

    feat16 = feat.astype(np.float16)
    # slot -> src node id / dst node id (0 for pad slots: their P row is all
    # zero so the scatter multiplies them by zero)
    se_ids = np.zeros((M, n_slots), dtype=np.int64)
    de_ids = np.zeros((M, n_slots), dtype=np.int64)
    drel_all = np.full((M, n_groups, P, GRP), -1.0, dtype=np.float16)

    for d in range(M):
        sd, dd, w, r = per_dev[d]
        if not len(dd):
            continue
        tau = w * K + r // P
        p = r % P
        slot = tau * P + p            # tile-major, partition-minor
        se_ids[d, slot] = sd
        de_ids[d, slot] = dd + d * NLOC
        g = tau // GRP
        t = tau % GRP
        drel_all[d, g, p, t] = (dd - w * P).astype(np.float16)

    feat_wins = []
    for d in range(M):
        fwin = np.zeros((OUT_ROWS, IN_FEATS), dtype=np.float16)
        fwin[:NLOC] = feat16[d * NLOC:(d + 1) * NLOC]
        feat_wins.append(fwin)

    attn_f = np.asarray(attn_e, np.float32).reshape(HF)
    attn_hf = attn_f.reshape(N_HEADS, OUT_FEATS)

    def aug(WT):
        y = WT * attn_f[None, :]
        u = 0.6 * np.einsum("ahf,hf->ah", WT.reshape(IN_FEATS, N_HEADS,
                                                     OUT_FEATS), attn_hf)
        return np.concatenate([y, u], axis=1).astype(np.float16)

    WsT = np.ascontiguousarray(np.asarray(W_src, np.float32).T)
    WdT = np.ascontiguousarray(np.asarray(W_dst, np.float32).T)
    cst = {
        "wsrcT": WsT.astype(np.float16),
        "wys": aug(WsT),
        "wyd": aug(WdT),
        "wresT": np.ascontiguousarray(np.asarray(W_res, np.float32).T).astype(np.float16),
        "sign04": np.tile((0.4 * np.sign(attn_f)).astype(np.float16)[None, :],
                          (P, GRP)),
    }

    gpp = (n_groups + NSPLIT - 1) // NSPLIT
    in_maps = []
    for d in range(M):
        m = dict(cst)
        # interleave per group: [se rows | de rows] in one block
        fse = feat16[se_ids[d]].reshape(n_groups, GRP * P, IN_FEATS)
        fde = feat16[de_ids[d]].reshape(n_groups, GRP * P, IN_FEATS)
        # one-hot P shipped transposed so the transpose-load yields P [e, n]:
        # row (t*128+n), col p = (drel[g,p,t] == n)
        dr = drel_all[d].astype(np.float32)          # [ng, P, GRP]
        oh = (dr.transpose(0, 2, 1)[:, :, :, None] ==
              np.arange(P, dtype=np.float32)[None, None, None, :])
        prows = np.ascontiguousarray(
            oh.transpose(0, 1, 3, 2)).astype(np.float16).reshape(
                n_groups, GRP * P, P)
        sede = np.concatenate([fse, fde, prows], axis=1)  # [ng, 3*GRP*P, D]
        for i in range(NSPLIT):
            g0 = gpp * i
            g1 = min(gpp * (i + 1), n_groups)
            m[f"feat_sede{i}"] = sede[g0:g1].reshape(-1, IN_FEATS)
        m["feat_win"] = feat_wins[d]
        in_maps.append(m)
    return K, n_groups, in_maps


def kernel(feat, W_src, b_src, W_dst, b_dst, attn_e, W_res, b_res, src, dst,
           _trace=False, _trace_kwargs=None):
    K, n_groups, in_maps = _preprocess(feat, W_src, b_src, W_dst, b_dst,
                                       attn_e, W_res, b_res, src, dst)
    key = (K, n_groups)
    if key not in _prog_cache:
        _prog_cache[key] = _build_program(K, n_groups)
    nc = _prog_cache[key]

    kw = {}
    if _trace:
        kw = dict(trace=True, trace_kwargs=_trace_kwargs or {})
    res = run_bass_kernel_spmd(nc, in_maps, core_ids=list(range(M)), **kw)
    outs = [res.results[d]["out_d"][:NLOC] for d in range(M)]
    full = np.concatenate(outs, axis=0).reshape(N_NODES, N_HEADS, OUT_FEATS)
    kernel._last_results = res
    kernel._last_cfg = (K, n_groups)
    return full
